# revision 28
# baseline (speedup 1.0000x reference)
"""SAGEConv (mean aggregation) + ReLU on 8 Trainium2 NeuronCores.

    out = relu( (mean_{j in N(i)} x_j) @ W_l.T + b_l + x_i @ W_r.T )

Strategy (graph/data parallel, hardcoded for N=100000, E=1600000, D=128):
  - Destination nodes are partitioned across 8 cores (12500 nodes each).
  - Edges are grouped by (core, 128-dst block, src chunk); source features
    are fetched with the Q7 `dma_gather` custom instruction. Chunks are
    [32768, 32768, 32768, 1696] rows — 32768 uses the int16 index range
    fully, so (block, chunk) groups are denser than with 4x25000: t4s ends
    up (6,6,6,1) -> 2432 gathered idxs/block instead of 2560, and 10 gather
    instructions per superblock instead of 12. This attacks the Q7
    descriptor-generation bottleneck directly: HW exec 2.217ms vs 2.294ms
    for the 4x25000 layout (reps-delta timing, rel err 5.07e-07).
  - Triple-buffered gather/meta pools (bufs=3) plus skipping the
    structurally-empty pad blocks 98/99 (dst rows >= 12544 > 12500) in the
    last superblock: HW exec 2.119ms (same rel err).
  - Per dst block, a scaled one-hot selection matrix S[e, d] =
    (dstrel[e]==d) * (1/deg[dst]) is built on the vector engine with one
    tensor_scalar(is_equal, mult) op per 128-edge tile, and the segment mean
    msgT[f, d] = sum_e Xg[e, f] * S[e, d] accumulates in PSUM on the tensor
    engine. out[d, :] = msgT.T @ W_l.T + x_loc @ W_r.T + b_l (three
    PSUM-accumulated matmuls, bias via a K=1 matmul with a ones row), then
    ReLU on the scalar engine. Weights/x chunks replicated; x_loc arrives
    pre-transposed per core.

Perf ledger (HW, NTFF traces; all engines other than GpSimd are hidden):
  - GPSIMD-bound: Q7 descriptor gen costs ~8ns/idx + ~0.6us/instruction at
    ~98% occupancy; DMA <50% busy, tensor ~45%, vector ~77% (fp32 S-build).
    Q7 gen is quantized at 128-idx chunks (dummy descriptors pushed for
    invalid lanes of a partial chunk).
  - Trailing -1 idx truncation (the documented padding-skip path) HANGS the
    device in every form tried: static num_idxs_reg=cap, per-count
    num_idxs_reg loaded from SBUF via gpsimd.load, per-(block,chunk) 640-idx
    instructions, per-(block-pair,chunk) 1280-idx instructions with counts
    always >= 640.
  - bf16 x does not reduce Q7 time (per-descriptor, not per-byte) and
    slightly worsens the gather slope; 400x640-idx instruction split with
    0-padding measures 2.78ms (per-instruction fixed cost).
  - Gathers of 1280/2048 idxs fault (SWDGE ring ~64-80 descs/engine);
    num_swdge_queues=4 with queue_num=q crashes unrecoverably; queues cannot
    parallelize gen anyway (8 Q7 cores run one BSP instruction at a time).
  - Falsified: tensor_scalar in0 from PSUM (breaks DVE 2-port mode, which
    docs say locks GpSimd out of its SBUF descriptor rings) leaves gather
    durations bit-identical — Q7 gen is deterministic intrinsic compute.
    (It does flatten DVE to 37% busy if headroom is ever needed.)
  - Wider dst blocks (256/512) scale the S-matmul's PSUM columns with block
    width; PE overtakes the gather saving. Not viable.
  - Remaining known path to go materially faster: executing host-precomputed
    descriptor blobs without Q7 (indices are static per instance), which
    bass does not expose for local HBM->SBUF gathers today.
"""

import math

import numpy as np

import concourse.bass as bass
import concourse.bacc as bacc
import concourse.mybir as mybir
import concourse.tile as tile
from concourse import library_config
from concourse.bass_utils import run_bass_kernel_spmd

N = 100000
E = 1600000
D = 128
NCORES = 8
NPC = N // NCORES
NB = 100
G = 4
NSB = NB // G
NQ = 4
CHS = [32768, 32768, 32768, N - 3 * 32768]  # chunk sizes (rows)
F32 = mybir.dt.float32
I16 = mybir.dt.int16


def _build_nc(t4s, reps=1, nsb=NSB):
    t4s = list(t4s)
    caps = [t * 128 for t in t4s]
    slots_b = sum(t4s)  # tiles per block
    slots_sb = G * slots_b
    qoff2 = np.concatenate([[0], np.cumsum(t4s)])  # dr/re col base per q
    qslot = np.concatenate([[0], np.cumsum([G * t for t in t4s])])  # xg slot base
    qidx = np.concatenate([[0], np.cumsum([G * c for c in caps])])  # idx pos base
    idxw_cols = int(qidx[-1]) // 16

    nc = bacc.Bacc("TRN2", target_bir_lowering=False, debug=False)
    xq = [
        nc.dram_tensor(f"x{q}", [CHS[q], D], F32, kind="ExternalInput")
        for q in range(NQ)
    ]
    nb = nsb * G
    idxs = nc.dram_tensor("idxs", [nsb, 128, idxw_cols], I16, kind="ExternalInput")
    dstrel = nc.dram_tensor("dstrel", [nsb, 128, slots_sb], F32, kind="ExternalInput")
    redge = nc.dram_tensor("redge", [nsb, 128, slots_sb], F32, kind="ExternalInput")
    iota = nc.dram_tensor("iota", [128, 128], F32, kind="ExternalInput")
    xloct = nc.dram_tensor("xloct", [128, nb * 128], F32, kind="ExternalInput")
    wlt = nc.dram_tensor("wlt", [D, D], F32, kind="ExternalInput")
    wrt = nc.dram_tensor("wrt", [D, D], F32, kind="ExternalInput")
    misc = nc.dram_tensor("misc", [2, D], F32, kind="ExternalInput")
    out = nc.dram_tensor("out", [nb * 128, D], F32, kind="ExternalOutput")

    with tile.TileContext(nc) as tc:
        with tc.tile_critical():
            nc.gpsimd.load_library(library_config.mlp)
        with (
            tc.tile_pool(name="const", bufs=1) as cpool,
            tc.tile_pool(name="xg", bufs=4) as xgpool,
            tc.tile_pool(name="meta", bufs=4) as mpool,
            tc.tile_pool(name="s", bufs=6) as spool,
            tc.tile_pool(name="work", bufs=3) as wpool,
            tc.tile_pool(name="psum", bufs=2, space="PSUM") as ppool,
        ):
            iota_sb = cpool.tile([128, 128], F32)
            nc.sync.dma_start(out=iota_sb[:], in_=iota[:])
            wlt_sb = cpool.tile([D, D], F32)
            nc.sync.dma_start(out=wlt_sb[:], in_=wlt[:])
            wrt_sb = cpool.tile([D, D], F32)
            nc.sync.dma_start(out=wrt_sb[:], in_=wrt[:])
            blr_sb = cpool.tile([1, D], F32)
            nc.sync.dma_start(out=blr_sb[:], in_=misc[0:1, :])
            ones_sb = cpool.tile([1, D], F32)
            nc.sync.dma_start(out=ones_sb[:], in_=misc[1:2, :])

            def body():
                for sb in range(nsb):
                    idx_sb = mpool.tile([128, idxw_cols], I16, tag="idx")
                    nc.sync.dma_start(out=idx_sb[:], in_=idxs[sb])
                    dr_sb = mpool.tile([128, slots_sb], F32, tag="dr")
                    nc.sync.dma_start(out=dr_sb[:], in_=dstrel[sb])
                    re_sb = mpool.tile([128, slots_sb], F32, tag="re")
                    nc.sync.dma_start(out=re_sb[:], in_=redge[sb])

                    xg = xgpool.tile([128, slots_sb * 128], F32, tag="xg")
                    # <=1024-idx gather pieces (HW ring limit), per chunk q.
                    # The last superblock's blocks 98/99 are structurally
                    # empty (dst rows >= 12544 > NPC): skip their gathers,
                    # matmuls and output entirely.
                    MAXS = 8
                    gr = G if sb < nsb - 1 else 2
                    for q in range(NQ):
                        nslot_q = gr * t4s[q]
                        for s0 in range(0, nslot_q, MAXS):
                            ns = min(MAXS, nslot_q - s0)
                            base = int(qslot[q]) + s0
                            nidx = ns * 128
                            c0 = (int(qidx[q]) + s0 * 128) // 16
                            nc.gpsimd.dma_gather(
                                xg[:, base * 128 : (base + ns) * 128].rearrange(
                                    "p (s d) -> p s d", d=128
                                ),
                                xq[q][:],
                                idx_sb[:, c0 : c0 + nidx // 16],
                                nidx,
                                nidx,
                                D,
                            )

                    for bi in range(gr):
                        b = sb * G + bi
                        msgt = ppool.tile([128, 128], F32, tag="msgt")
                        for j in range(slots_b):
                            q = int(np.searchsorted(qoff2, j, side="right")) - 1
                            t = j - int(qoff2[q])
                            sl = int(qslot[q]) + bi * t4s[q] + t
                            col = bi * slots_b + j
                            s_t = spool.tile([128, 128], F32, tag="s")
                            nc.vector.tensor_scalar(
                                out=s_t[:],
                                in0=iota_sb[:],
                                scalar1=dr_sb[:, col : col + 1],
                                scalar2=re_sb[:, col : col + 1],
                                op0=mybir.AluOpType.is_equal,
                                op1=mybir.AluOpType.mult,
                            )
                            nc.tensor.matmul(
                                out=msgt[:],
                                lhsT=xg[:, sl * 128 : (sl + 1) * 128],
                                rhs=s_t[:],
                                start=(j == 0),
                                stop=(j == slots_b - 1),
                            )
                        aggt = wpool.tile([128, 128], F32, tag="aggt")
                        nc.vector.tensor_copy(out=aggt[:], in_=msgt[:])
                        xct = wpool.tile([128, 128], F32, tag="xct")
                        nc.sync.dma_start(out=xct[:], in_=xloct[:, b * 128 : (b + 1) * 128])
                        outp = ppool.tile([128, D], F32, tag="outp")
                        nc.tensor.matmul(out=outp[:], lhsT=aggt[:], rhs=wlt_sb[:], start=True, stop=False)
                        nc.tensor.matmul(out=outp[:], lhsT=xct[:], rhs=wrt_sb[:], start=False, stop=False)
                        nc.tensor.matmul(out=outp[:], lhsT=ones_sb[:], rhs=blr_sb[:], start=False, stop=True)
                        outs = wpool.tile([128, D], F32, tag="outs")
                        nc.scalar.activation(outs[:], outp[:], mybir.ActivationFunctionType.Relu)
                        nc.sync.dma_start(out=out[b * 128 : (b + 1) * 128, :], in_=outs[:])

            if reps == 1:
                body()
            else:
                with tc.For_i(0, reps, 1):
                    body()
    nc.compile()
    return nc


def _prep(x, edge_index):
    """Host-side sharding with variable-size src chunks."""
    x = np.ascontiguousarray(np.asarray(x, dtype=np.float32))
    src = np.asarray(edge_index[0], dtype=np.int64)
    dst = np.asarray(edge_index[1], dtype=np.int64)

    deg = np.bincount(dst, minlength=N)
    rec = (1.0 / np.maximum(deg, 1.0)).astype(np.float32)

    c = dst // NPC
    local = dst - c * NPC
    b = local >> 7
    drel = (local & 127).astype(np.float32)
    q = (src >> 15).astype(np.int64)  # chunk boundaries at 32768 multiples
    i16 = (src - (q << 15)).astype(np.int16)
    re = rec[dst]

    nkey = NCORES * NB
    key = c * NB + b

    t4s, idxq_l, drq_l, req_l = [], [], [], []
    for qq in range(NQ):
        sel = q == qq
        keyq = key[sel]
        counts = np.bincount(keyq, minlength=nkey)
        t4 = max(1, math.ceil(counts.max() / 128))
        cap = t4 * 128
        order = np.argsort(keyq, kind="stable")
        keys = keyq[order]
        starts = np.zeros(nkey, np.int64)
        np.cumsum(counts[:-1], out=starts[1:])
        pos = np.arange(len(keys), dtype=np.int64) - starts[keys]
        destq = keys * cap + pos
        tot = nkey * cap
        idxp = np.zeros(tot, np.int16)
        drp = np.full(tot, -1.0, np.float32)
        rep = np.zeros(tot, np.float32)
        idxp[destq] = i16[sel][order]
        drp[destq] = drel[sel][order]
        rep[destq] = re[sel][order]
        t4s.append(t4)
        idxq_l.append(idxp.reshape(NCORES, NSB, G, cap))
        drq_l.append(drp.reshape(NCORES, NSB, G, t4, 128))
        req_l.append(rep.reshape(NCORES, NSB, G, t4, 128))

    slots_b = sum(t4s)
    # idxw: per (sb, q) wrap G*cap idxs (idx i -> [i%16, i//16]), replicate
    # to 128 partitions, concat chunks along columns.
    wraps = []
    for qq in range(NQ):
        cap = t4s[qq] * 128
        flat = idxq_l[qq].reshape(NCORES, NSB, G * cap)
        w = flat.reshape(NCORES, NSB, G * cap // 16, 16).transpose(0, 1, 3, 2)
        wraps.append(np.tile(w[:, :, None, :, :], (1, 1, 1, 8, 1)).reshape(
            NCORES, NSB, 128, G * cap // 16))
    idxw = np.ascontiguousarray(np.concatenate(wraps, axis=3))

    # dstrel/redge: [c, NSB, 128, G*slots_b], col = bi*slots_b + qoff2[q] + t
    dr_dev = np.zeros((NCORES, NSB, 128, G, slots_b), np.float32)
    re_dev = np.zeros((NCORES, NSB, 128, G, slots_b), np.float32)
    off = 0
    for qq in range(NQ):
        t4 = t4s[qq]
        dr_dev[..., off : off + t4] = drq_l[qq].transpose(0, 1, 4, 2, 3)
        re_dev[..., off : off + t4] = req_l[qq].transpose(0, 1, 4, 2, 3)
        off += t4
    dr_dev = np.ascontiguousarray(dr_dev.reshape(NCORES, NSB, 128, G * slots_b))
    re_dev = np.ascontiguousarray(re_dev.reshape(NCORES, NSB, 128, G * slots_b))

    bounds = np.concatenate([[0], np.cumsum(CHS)])
    xq_np = [x[bounds[qq] : bounds[qq + 1]] for qq in range(NQ)]

    xloct = np.zeros((NCORES, 128, NB * 128), np.float32)
    for cc in range(NCORES):
        xl = np.zeros((NB * 128, D), np.float32)
        xl[:NPC] = x[cc * NPC : (cc + 1) * NPC]
        xloct[cc] = xl.T

    return tuple(t4s), xq_np, idxw, dr_dev, re_dev, xloct


def _in_maps(inputs):
    x = inputs["x"]
    edge_index = inputs["edge_index"]
    w_l = np.asarray(inputs["W_l"], dtype=np.float32)
    b_l = np.asarray(inputs["b_l"], dtype=np.float32)
    w_r = np.asarray(inputs["W_r"], dtype=np.float32)

    t4s, xq_np, idxw, dr_dev, re_dev, xloct = _prep(x, edge_index)

    iota_np = np.ascontiguousarray(
        np.broadcast_to(np.arange(128, dtype=np.float32), (128, 128))
    )
    wlt_np = np.ascontiguousarray(w_l.T)
    wrt_np = np.ascontiguousarray(w_r.T)
    misc_np = np.stack([b_l, np.ones(D, np.float32)])

    in_maps = []
    for c in range(NCORES):
        m = {f"x{q}": xq_np[q] for q in range(NQ)}
        m.update(
            idxs=idxw[c], dstrel=dr_dev[c], redge=re_dev[c], iota=iota_np,
            xloct=xloct[c], wlt=wlt_np, wrt=wrt_np, misc=misc_np,
        )
        in_maps.append(m)
    return t4s, in_maps


def _run(inputs, reps=1):
    t4s, in_maps = _in_maps(inputs)
    nc = _build_nc(t4s, reps=reps)
    res = run_bass_kernel_spmd(nc, in_maps, core_ids=list(range(NCORES)))
    out = np.concatenate(
        [res.results[c]["out"][:NPC] for c in range(NCORES)], axis=0
    )
    return out


def kernel(**inputs) -> np.ndarray:
    return _run(inputs, reps=1)


# revision 29
# speedup vs baseline: 1.0004x; 1.0004x over previous
"""SAGEConv (mean aggregation) + ReLU on 8 Trainium2 NeuronCores.

    out = relu( (mean_{j in N(i)} x_j) @ W_l.T + b_l + x_i @ W_r.T )

Strategy (graph/data parallel, hardcoded for N=100000, E=1600000, D=128):
  - Destination nodes are partitioned across 8 cores (12500 nodes each).
  - Edges are grouped by (core, 128-dst block, src chunk); source features
    are fetched with the Q7 `dma_gather` custom instruction. Chunks are
    [32768, 32768, 32768, 1696] rows — 32768 uses the int16 index range
    fully, so (block, chunk) groups are denser than with 4x25000: t4s ends
    up (6,6,6,1) -> 2432 gathered idxs/block instead of 2560, and 10 gather
    instructions per superblock instead of 12. This attacks the Q7
    descriptor-generation bottleneck directly: HW exec 2.217ms vs 2.294ms
    for the 4x25000 layout (reps-delta timing, rel err 5.07e-07).
  - Triple-buffered gather/meta pools (bufs=3) plus skipping the
    structurally-empty pad blocks 98/99 (dst rows >= 12544 > 12500) in the
    last superblock: HW exec 2.119ms (same rel err).
  - Per dst block, a scaled one-hot selection matrix S[e, d] =
    (dstrel[e]==d) * (1/deg[dst]) is built on the vector engine with one
    tensor_scalar(is_equal, mult) op per 128-edge tile, and the segment mean
    msgT[f, d] = sum_e Xg[e, f] * S[e, d] accumulates in PSUM on the tensor
    engine. out[d, :] = msgT.T @ W_l.T + x_loc @ W_r.T + b_l (three
    PSUM-accumulated matmuls, bias via a K=1 matmul with a ones row), then
    ReLU on the scalar engine. Weights/x chunks replicated; x_loc arrives
    pre-transposed per core.

Perf ledger (HW, NTFF traces; all engines other than GpSimd are hidden):
  - GPSIMD-bound: Q7 descriptor gen costs ~8ns/idx + ~0.6us/instruction at
    ~98% occupancy; DMA <50% busy, tensor ~45%, vector ~77% (fp32 S-build).
    Q7 gen is quantized at 128-idx chunks (dummy descriptors pushed for
    invalid lanes of a partial chunk).
  - Trailing -1 idx truncation (the documented padding-skip path) HANGS the
    device in every form tried: static num_idxs_reg=cap, per-count
    num_idxs_reg loaded from SBUF via gpsimd.load, per-(block,chunk) 640-idx
    instructions, per-(block-pair,chunk) 1280-idx instructions with counts
    always >= 640.
  - bf16 x does not reduce Q7 time (per-descriptor, not per-byte) and
    slightly worsens the gather slope; 400x640-idx instruction split with
    0-padding measures 2.78ms (per-instruction fixed cost).
  - Gathers of 1280/2048 idxs fault (SWDGE ring ~64-80 descs/engine);
    num_swdge_queues=4 with queue_num=q crashes unrecoverably; queues cannot
    parallelize gen anyway (8 Q7 cores run one BSP instruction at a time).
  - Falsified: tensor_scalar in0 from PSUM (breaks DVE 2-port mode, which
    docs say locks GpSimd out of its SBUF descriptor rings) leaves gather
    durations bit-identical — Q7 gen is deterministic intrinsic compute.
    (It does flatten DVE to 37% busy if headroom is ever needed.)
  - Wider dst blocks (256/512) scale the S-matmul's PSUM columns with block
    width; PE overtakes the gather saving. Not viable.
  - Remaining known path to go materially faster: executing host-precomputed
    descriptor blobs without Q7 (indices are static per instance), which
    bass does not expose for local HBM->SBUF gathers today.
"""

import math

import numpy as np

import concourse.bass as bass
import concourse.bacc as bacc
import concourse.mybir as mybir
import concourse.tile as tile
from concourse import library_config
from concourse.bass_utils import run_bass_kernel_spmd

N = 100000
E = 1600000
D = 128
NCORES = 8
NPC = N // NCORES
NB = 100
G = 4
NSB = NB // G
NQ = 4
CHS = [32768, 32768, 32768, N - 3 * 32768]  # chunk sizes (rows)
F32 = mybir.dt.float32
I16 = mybir.dt.int16


def _build_nc(t4s, reps=1, nsb=NSB):
    t4s = list(t4s)
    caps = [t * 128 for t in t4s]
    slots_b = sum(t4s)  # tiles per block
    slots_sb = G * slots_b
    qoff2 = np.concatenate([[0], np.cumsum(t4s)])  # dr/re col base per q
    qslot = np.concatenate([[0], np.cumsum([G * t for t in t4s])])  # xg slot base
    qidx = np.concatenate([[0], np.cumsum([G * c for c in caps])])  # idx pos base
    idxw_cols = int(qidx[-1]) // 16

    nc = bacc.Bacc("TRN2", target_bir_lowering=False, debug=False)
    xq = [
        nc.dram_tensor(f"x{q}", [CHS[q], D], F32, kind="ExternalInput")
        for q in range(NQ)
    ]
    nb = nsb * G
    idxs = nc.dram_tensor("idxs", [nsb, 128, idxw_cols], I16, kind="ExternalInput")
    dstrel = nc.dram_tensor("dstrel", [nsb, 128, slots_sb], F32, kind="ExternalInput")
    redge = nc.dram_tensor("redge", [nsb, 128, slots_sb], F32, kind="ExternalInput")
    iota = nc.dram_tensor("iota", [128, 128], F32, kind="ExternalInput")
    xloct = nc.dram_tensor("xloct", [128, nb * 128], F32, kind="ExternalInput")
    wlt = nc.dram_tensor("wlt", [D, D], F32, kind="ExternalInput")
    wrt = nc.dram_tensor("wrt", [D, D], F32, kind="ExternalInput")
    misc = nc.dram_tensor("misc", [2, D], F32, kind="ExternalInput")
    out = nc.dram_tensor("out", [nb * 128, D], F32, kind="ExternalOutput")

    with tile.TileContext(nc) as tc:
        with tc.tile_critical():
            nc.gpsimd.load_library(library_config.mlp)
        with (
            tc.tile_pool(name="const", bufs=1) as cpool,
            tc.tile_pool(name="xg", bufs=3) as xgpool,
            tc.tile_pool(name="meta", bufs=3) as mpool,
            tc.tile_pool(name="s", bufs=6) as spool,
            tc.tile_pool(name="work", bufs=3) as wpool,
            tc.tile_pool(name="psum", bufs=2, space="PSUM") as ppool,
        ):
            iota_sb = cpool.tile([128, 128], F32)
            nc.sync.dma_start(out=iota_sb[:], in_=iota[:])
            wlt_sb = cpool.tile([D, D], F32)
            nc.sync.dma_start(out=wlt_sb[:], in_=wlt[:])
            wrt_sb = cpool.tile([D, D], F32)
            nc.sync.dma_start(out=wrt_sb[:], in_=wrt[:])
            blr_sb = cpool.tile([1, D], F32)
            nc.sync.dma_start(out=blr_sb[:], in_=misc[0:1, :])
            ones_sb = cpool.tile([1, D], F32)
            nc.sync.dma_start(out=ones_sb[:], in_=misc[1:2, :])

            def body():
                for sb in range(nsb):
                    idx_sb = mpool.tile([128, idxw_cols], I16, tag="idx")
                    nc.sync.dma_start(out=idx_sb[:], in_=idxs[sb])
                    dr_sb = mpool.tile([128, slots_sb], F32, tag="dr")
                    nc.sync.dma_start(out=dr_sb[:], in_=dstrel[sb])
                    re_sb = mpool.tile([128, slots_sb], F32, tag="re")
                    nc.sync.dma_start(out=re_sb[:], in_=redge[sb])

                    xg = xgpool.tile([128, slots_sb * 128], F32, tag="xg")
                    # <=1024-idx gather pieces (HW ring limit), per chunk q.
                    # The last superblock's blocks 98/99 are structurally
                    # empty (dst rows >= 12544 > NPC): skip their gathers,
                    # matmuls and output entirely.
                    MAXS = 8
                    gr = G if sb < nsb - 1 else 2
                    for q in range(NQ):
                        nslot_q = gr * t4s[q]
                        for s0 in range(0, nslot_q, MAXS):
                            ns = min(MAXS, nslot_q - s0)
                            base = int(qslot[q]) + s0
                            nidx = ns * 128
                            c0 = (int(qidx[q]) + s0 * 128) // 16
                            nc.gpsimd.dma_gather(
                                xg[:, base * 128 : (base + ns) * 128].rearrange(
                                    "p (s d) -> p s d", d=128
                                ),
                                xq[q][:],
                                idx_sb[:, c0 : c0 + nidx // 16],
                                nidx,
                                nidx,
                                D,
                            )

                    for bi in range(gr):
                        b = sb * G + bi
                        msgt = ppool.tile([128, 128], F32, tag="msgt")
                        for j in range(slots_b):
                            q = int(np.searchsorted(qoff2, j, side="right")) - 1
                            t = j - int(qoff2[q])
                            sl = int(qslot[q]) + bi * t4s[q] + t
                            col = bi * slots_b + j
                            s_t = spool.tile([128, 128], F32, tag="s")
                            nc.vector.tensor_scalar(
                                out=s_t[:],
                                in0=iota_sb[:],
                                scalar1=dr_sb[:, col : col + 1],
                                scalar2=re_sb[:, col : col + 1],
                                op0=mybir.AluOpType.is_equal,
                                op1=mybir.AluOpType.mult,
                            )
                            nc.tensor.matmul(
                                out=msgt[:],
                                lhsT=xg[:, sl * 128 : (sl + 1) * 128],
                                rhs=s_t[:],
                                start=(j == 0),
                                stop=(j == slots_b - 1),
                            )
                        aggt = wpool.tile([128, 128], F32, tag="aggt")
                        nc.vector.tensor_copy(out=aggt[:], in_=msgt[:])
                        xct = wpool.tile([128, 128], F32, tag="xct")
                        nc.sync.dma_start(out=xct[:], in_=xloct[:, b * 128 : (b + 1) * 128])
                        outp = ppool.tile([128, D], F32, tag="outp")
                        nc.tensor.matmul(out=outp[:], lhsT=aggt[:], rhs=wlt_sb[:], start=True, stop=False)
                        nc.tensor.matmul(out=outp[:], lhsT=xct[:], rhs=wrt_sb[:], start=False, stop=False)
                        nc.tensor.matmul(out=outp[:], lhsT=ones_sb[:], rhs=blr_sb[:], start=False, stop=True)
                        outs = wpool.tile([128, D], F32, tag="outs")
                        nc.scalar.activation(outs[:], outp[:], mybir.ActivationFunctionType.Relu)
                        nc.sync.dma_start(out=out[b * 128 : (b + 1) * 128, :], in_=outs[:])

            if reps == 1:
                body()
            else:
                with tc.For_i(0, reps, 1):
                    body()
    nc.compile()
    return nc


def _prep(x, edge_index):
    """Host-side sharding with variable-size src chunks."""
    x = np.ascontiguousarray(np.asarray(x, dtype=np.float32))
    src = np.asarray(edge_index[0], dtype=np.int64)
    dst = np.asarray(edge_index[1], dtype=np.int64)

    deg = np.bincount(dst, minlength=N)
    rec = (1.0 / np.maximum(deg, 1.0)).astype(np.float32)

    c = dst // NPC
    local = dst - c * NPC
    b = local >> 7
    drel = (local & 127).astype(np.float32)
    q = (src >> 15).astype(np.int64)  # chunk boundaries at 32768 multiples
    i16 = (src - (q << 15)).astype(np.int16)
    re = rec[dst]

    nkey = NCORES * NB
    key = c * NB + b

    t4s, idxq_l, drq_l, req_l = [], [], [], []
    for qq in range(NQ):
        sel = q == qq
        keyq = key[sel]
        counts = np.bincount(keyq, minlength=nkey)
        t4 = max(1, math.ceil(counts.max() / 128))
        cap = t4 * 128
        order = np.argsort(keyq, kind="stable")
        keys = keyq[order]
        starts = np.zeros(nkey, np.int64)
        np.cumsum(counts[:-1], out=starts[1:])
        pos = np.arange(len(keys), dtype=np.int64) - starts[keys]
        destq = keys * cap + pos
        tot = nkey * cap
        idxp = np.zeros(tot, np.int16)
        drp = np.full(tot, -1.0, np.float32)
        rep = np.zeros(tot, np.float32)
        idxp[destq] = i16[sel][order]
        drp[destq] = drel[sel][order]
        rep[destq] = re[sel][order]
        t4s.append(t4)
        idxq_l.append(idxp.reshape(NCORES, NSB, G, cap))
        drq_l.append(drp.reshape(NCORES, NSB, G, t4, 128))
        req_l.append(rep.reshape(NCORES, NSB, G, t4, 128))

    slots_b = sum(t4s)
    # idxw: per (sb, q) wrap G*cap idxs (idx i -> [i%16, i//16]), replicate
    # to 128 partitions, concat chunks along columns.
    wraps = []
    for qq in range(NQ):
        cap = t4s[qq] * 128
        flat = idxq_l[qq].reshape(NCORES, NSB, G * cap)
        w = flat.reshape(NCORES, NSB, G * cap // 16, 16).transpose(0, 1, 3, 2)
        wraps.append(np.tile(w[:, :, None, :, :], (1, 1, 1, 8, 1)).reshape(
            NCORES, NSB, 128, G * cap // 16))
    idxw = np.ascontiguousarray(np.concatenate(wraps, axis=3))

    # dstrel/redge: [c, NSB, 128, G*slots_b], col = bi*slots_b + qoff2[q] + t
    dr_dev = np.zeros((NCORES, NSB, 128, G, slots_b), np.float32)
    re_dev = np.zeros((NCORES, NSB, 128, G, slots_b), np.float32)
    off = 0
    for qq in range(NQ):
        t4 = t4s[qq]
        dr_dev[..., off : off + t4] = drq_l[qq].transpose(0, 1, 4, 2, 3)
        re_dev[..., off : off + t4] = req_l[qq].transpose(0, 1, 4, 2, 3)
        off += t4
    dr_dev = np.ascontiguousarray(dr_dev.reshape(NCORES, NSB, 128, G * slots_b))
    re_dev = np.ascontiguousarray(re_dev.reshape(NCORES, NSB, 128, G * slots_b))

    bounds = np.concatenate([[0], np.cumsum(CHS)])
    xq_np = [x[bounds[qq] : bounds[qq + 1]] for qq in range(NQ)]

    xloct = np.zeros((NCORES, 128, NB * 128), np.float32)
    for cc in range(NCORES):
        xl = np.zeros((NB * 128, D), np.float32)
        xl[:NPC] = x[cc * NPC : (cc + 1) * NPC]
        xloct[cc] = xl.T

    return tuple(t4s), xq_np, idxw, dr_dev, re_dev, xloct


def _in_maps(inputs):
    x = inputs["x"]
    edge_index = inputs["edge_index"]
    w_l = np.asarray(inputs["W_l"], dtype=np.float32)
    b_l = np.asarray(inputs["b_l"], dtype=np.float32)
    w_r = np.asarray(inputs["W_r"], dtype=np.float32)

    t4s, xq_np, idxw, dr_dev, re_dev, xloct = _prep(x, edge_index)

    iota_np = np.ascontiguousarray(
        np.broadcast_to(np.arange(128, dtype=np.float32), (128, 128))
    )
    wlt_np = np.ascontiguousarray(w_l.T)
    wrt_np = np.ascontiguousarray(w_r.T)
    misc_np = np.stack([b_l, np.ones(D, np.float32)])

    in_maps = []
    for c in range(NCORES):
        m = {f"x{q}": xq_np[q] for q in range(NQ)}
        m.update(
            idxs=idxw[c], dstrel=dr_dev[c], redge=re_dev[c], iota=iota_np,
            xloct=xloct[c], wlt=wlt_np, wrt=wrt_np, misc=misc_np,
        )
        in_maps.append(m)
    return t4s, in_maps


def _run(inputs, reps=1):
    t4s, in_maps = _in_maps(inputs)
    nc = _build_nc(t4s, reps=reps)
    res = run_bass_kernel_spmd(nc, in_maps, core_ids=list(range(NCORES)))
    out = np.concatenate(
        [res.results[c]["out"][:NPC] for c in range(NCORES)], axis=0
    )
    return out


def kernel(**inputs) -> np.ndarray:
    return _run(inputs, reps=1)
